# revision 13
# baseline (speedup 1.0000x reference)
"""Trainium2 Bass kernel for nn_CombineLayer (gnn_message_passing).

Math: out_e = z[i0]@wo1 + z[i1]@v2 + z0[max(i0,i1)-M]@v3 + bo, where
wo1 = Wo[0,:128], v2 = Wq[:,:128].T @ Wo[0,128:], v3 = Wq[:,128:].T @ Wo[0,128:]
and M = min_e max(i0,i1). (Exact rewrite of the reference's two linears.)

Device work per core (edges sharded 8 ways, node tables sharded 8 ways):
  1. tables: a = z@wo1, b = z@v2, c = z0_shift@v3 over this core's node
     shard (z passed transposed so matmuls need no on-chip transpose),
     assembled into 4 segments [a+bo, a+c+bo, b, b+c].
  2. AllGather the 8 table shards -> full table G (400k entries).
  3. per edge: out = G[pos0] + G[pos1] via 4096-index indirect DMA
     gathers; the second stream accumulates with the DMA CCE adder.
Host does only integer index prep / layout (sharding, transpose,
position computation), all float math runs on device.
"""
import sys

sys.path.insert(0, "/opt/trn_rl_repo")

import numpy as np

N = 100000
E = 1000000
EMB = 128
NCORES = 8

S = 12500            # nodes per core (table shard)
SP = 12544           # padded to 128*98
TT = 98              # node tiles of 128 per core
SEG = 4              # table segments per core
RANK_BLK = SEG * SP  # 50176 table entries per rank
TBL = NCORES * RANK_BLK  # 401408

EPC = 125000         # edges per core
NIDX = 4096          # indices per gather instruction
CHUNKS = 31          # ceil(EPC / NIDX)
EPAD = CHUNKS * NIDX  # 126976


def _build_nc():
    import concourse.bass as bass
    import concourse.bacc as bacc
    import concourse.mybir as mybir
    import concourse.tile as tile

    nc = bacc.Bacc("TRN2", target_bir_lowering=False, debug=False,
                   num_devices=NCORES)
    f32 = mybir.dt.float32
    zT = nc.dram_tensor("zT", [EMB, SP], f32, kind="ExternalInput").ap()
    z0T = nc.dram_tensor("z0T", [EMB, SP], f32, kind="ExternalInput").ap()
    Wq = nc.dram_tensor("Wq", [EMB, 2 * EMB], f32, kind="ExternalInput").ap()
    Wo = nc.dram_tensor("Wo", [1, 2 * EMB], f32, kind="ExternalInput").ap()
    bo = nc.dram_tensor("bo", [1, 1], f32, kind="ExternalInput").ap()
    idx0 = nc.dram_tensor("idx0", [128, 32 * CHUNKS], mybir.dt.int32,
                          kind="ExternalInput").ap()
    idx1 = nc.dram_tensor("idx1", [128, 32 * CHUNKS], mybir.dt.int32,
                          kind="ExternalInput").ap()
    out = nc.dram_tensor("out", [CHUNKS, NIDX], f32,
                         kind="ExternalOutput").ap()

    with tile.TileContext(nc) as tc:
        with (
            tc.tile_pool(name="sb", bufs=2) as sb,
            tc.tile_pool(name="ps", bufs=2, space="PSUM") as ps,
            tc.tile_pool(name="dram", bufs=1, space="DRAM") as dp,
            tc.tile_pool(name="tbl", bufs=1) as tp,
        ):
            # ---- load weights / build projection vectors ----
            wq_s = sb.tile([128, 2 * EMB], f32, bufs=1)
            nc.sync.dma_start(wq_s[:], Wq[:])
            # wo columns: Wo[0, 0:128] and Wo[0, 128:256] as [128, 1]
            wo1_s = sb.tile([128, 1], f32, bufs=1)
            nc.sync.dma_start(
                wo1_s[:], Wo[0:1, 0:EMB].rearrange("a (p b) -> (a p) b", b=1))
            wo2_s = sb.tile([128, 1], f32, bufs=1)
            nc.sync.dma_start(
                wo2_s[:], Wo[0:1, EMB:2 * EMB].rearrange("a (p b) -> (a p) b",
                                                         b=1))
            bo_s = sb.tile([1, 1], f32, bufs=1)
            nc.sync.dma_start(bo_s[:], bo[:])
            bo128 = sb.tile([128, 1], f32, bufs=1)
            nc.gpsimd.partition_broadcast(bo128[:], bo_s[:])

            v2_p = ps.tile([128, 1], f32)
            nc.tensor.matmul(v2_p[:], wq_s[:, 0:EMB], wo2_s[:],
                             start=True, stop=True)
            v3_p = ps.tile([128, 1], f32)
            nc.tensor.matmul(v3_p[:], wq_s[:, EMB:2 * EMB], wo2_s[:],
                             start=True, stop=True)
            # W_ab = [wo1 | v2], W_c = [v3]
            w_ab = sb.tile([128, 2], f32, bufs=1)
            nc.vector.tensor_copy(w_ab[:, 0:1], wo1_s[:])
            nc.vector.tensor_copy(w_ab[:, 1:2], v2_p[:])
            w_c = sb.tile([128, 1], f32, bufs=1)
            nc.vector.tensor_copy(w_c[:], v3_p[:])

            # ---- node tables for this core's shard ----
            ab_p = ps.tile([128, 2 * TT], f32, bufs=1)   # a,b interleaved
            c_p = ps.tile([128, TT], f32, bufs=1)
            CW = 14  # node tiles per load chunk
            for c in range(TT // CW):
                zc = tp.tile([128, 128 * CW], f32, bufs=2, tag="zz")
                nc.sync.dma_start(zc[:], zT[:, 128 * CW * c:128 * CW * (c + 1)])
                for j in range(CW):
                    t = c * CW + j
                    nc.tensor.matmul(ab_p[:, 2 * t:2 * t + 2],
                                     zc[:, 128 * j:128 * (j + 1)], w_ab[:],
                                     start=True, stop=True)
                z0c = tp.tile([128, 128 * CW], f32, bufs=2, tag="zz")
                nc.sync.dma_start(z0c[:],
                                  z0T[:, 128 * CW * c:128 * CW * (c + 1)])
                for j in range(CW):
                    t = c * CW + j
                    nc.tensor.matmul(c_p[:, t:t + 1],
                                     z0c[:, 128 * j:128 * (j + 1)], w_c[:],
                                     start=True, stop=True)

            a_view = ab_p[:].rearrange("p (t two) -> p t two", two=2)[:, :, 0:1]
            b_view = ab_p[:].rearrange("p (t two) -> p t two", two=2)[:, :, 1:2]
            seg1a = sb.tile([128, TT], f32, bufs=1)
            nc.vector.tensor_scalar_add(
                seg1a[:].rearrange("p (t o) -> p t o", o=1), a_view, bo128[:, 0:1])
            seg1b = sb.tile([128, TT], f32, bufs=1)
            nc.vector.tensor_add(seg1b[:], seg1a[:], c_p[:])
            seg2a = sb.tile([128, TT], f32, bufs=1)
            nc.vector.tensor_copy(
                seg2a[:].rearrange("p (t o) -> p t o", o=1), b_view)
            seg2b = sb.tile([128, TT], f32, bufs=1)
            nc.vector.tensor_add(seg2b[:], seg2a[:], c_p[:])

            # ---- AllGather table shards ----
            ag_in = dp.tile([1, RANK_BLK], f32)
            for si, seg in enumerate([seg1a, seg1b, seg2a, seg2b]):
                nc.sync.dma_start(
                    ag_in[0:1, si * SP:(si + 1) * SP].rearrange(
                        "a (p t) -> (a p) t", p=128),
                    seg[:])
            g_tbl = dp.tile([1, TBL], f32, addr_space="Shared")
            nc.gpsimd.collective_compute(
                "AllGather",
                mybir.AluOpType.bypass,
                replica_groups=[list(range(NCORES))],
                ins=[ag_in.opt()],
                outs=[g_tbl.opt()],
            )
            g_src = g_tbl[:].rearrange("a (n b) -> (a n) b", b=1)

            # ---- edge gathers: out = G[pos0] + G[pos1] ----
            it0 = sb.tile([128, 32 * CHUNKS], mybir.dt.int32, bufs=1)
            nc.sync.dma_start(it0[:], idx0[:])
            it1 = sb.tile([128, 32 * CHUNKS], mybir.dt.int32, bufs=1)
            nc.sync.dma_start(it1[:], idx1[:])
            for ch in range(CHUNKS):
                g0 = sb.tile([1, NIDX], f32, tag="g0", bufs=4)
                nc.gpsimd.indirect_dma_start(
                    out=g0[:].rearrange("p (a b) -> p a b", b=1),
                    out_offset=None, in_=g_src,
                    in_offset=bass.IndirectOffsetOnAxis(
                        ap=it0[:, 32 * ch:32 * ch + 32], axis=0))
                g1 = sb.tile([1, NIDX], f32, tag="g1", bufs=4)
                nc.gpsimd.indirect_dma_start(
                    out=g1[:].rearrange("p (a b) -> p a b", b=1),
                    out_offset=None, in_=g_src,
                    in_offset=bass.IndirectOffsetOnAxis(
                        ap=it1[:, 32 * ch:32 * ch + 32], axis=0))
                go = sb.tile([1, NIDX], f32, tag="go", bufs=3)
                nc.vector.tensor_add(go[:], g0[:], g1[:])
                nc.sync.dma_start(out[ch:ch + 1, :], go[:])
    nc.compile()
    return nc


def kernel(z, edge_index, z0, Wq, Wo, bo):
    z = np.asarray(z, dtype=np.float32)
    z0 = np.asarray(z0, dtype=np.float32)
    Wq = np.asarray(Wq, dtype=np.float32)
    Wo = np.asarray(Wo, dtype=np.float32)
    bo = np.asarray(bo, dtype=np.float32)
    ei = np.asarray(edge_index)

    i0 = ei[0].astype(np.int64)
    i1 = ei[1].astype(np.int64)
    m = np.maximum(i0, i1)
    M = int(m.min())

    # table positions (int index preprocessing only)
    def pos_of(i_end, seg_pair_base, shifted):
        k = i_end // S
        n_loc = i_end - k * S
        t = n_loc // 128
        p = n_loc - t * 128
        seg = seg_pair_base + shifted
        return k * RANK_BLK + seg * SP + p * TT + t

    sh0 = (i0 >= i1).astype(np.int64)     # table1 shifted half used
    sh1 = 1 - sh0                         # table2 shifted half used
    pos0 = pos_of(i0, 0, sh0).astype(np.int32)
    pos1 = pos_of(i1, 2, sh1).astype(np.int32)

    # per-core inputs
    in_maps = []
    for k in range(NCORES):
        # node shard rows [k*S, k*S+SP) of z; z0 shifted window rows - M
        zpad = np.zeros((SP, EMB), dtype=np.float32)
        lo, hi = k * S, min(k * S + SP, N)
        zpad[:hi - lo] = z[lo:hi]
        z0pad = np.zeros((SP, EMB), dtype=np.float32)
        wlo, whi = k * S - M, k * S + SP - M
        slo, shi = max(wlo, 0), min(whi, N)
        if shi > slo:
            z0pad[slo - wlo:shi - wlo] = z0[slo:shi]
        p0 = np.zeros(EPAD, dtype=np.int32)
        p1 = np.zeros(EPAD, dtype=np.int32)
        p0[:EPC] = pos0[k * EPC:(k + 1) * EPC]
        p1[:EPC] = pos1[k * EPC:(k + 1) * EPC]
        # snake layout: instruction ch consumes idx[p, 32ch + c] with
        # linear order i = c*128 + p
        it0 = p0.reshape(CHUNKS, 32, 128).transpose(2, 0, 1).reshape(
            128, CHUNKS * 32)
        it1 = p1.reshape(CHUNKS, 32, 128).transpose(2, 0, 1).reshape(
            128, CHUNKS * 32)
        in_maps.append({
            "zT": np.ascontiguousarray(zpad.T),
            "z0T": np.ascontiguousarray(z0pad.T),
            "Wq": Wq, "Wo": Wo, "bo": bo.reshape(1, 1),
            "idx0": np.ascontiguousarray(it0),
            "idx1": np.ascontiguousarray(it1),
        })

    from concourse.bass_utils import run_bass_kernel_spmd
    nc = _build_nc()
    res = run_bass_kernel_spmd(nc, in_maps, core_ids=list(range(NCORES)))
    outs = []
    for k in range(NCORES):
        outs.append(res.results[k]["out"].reshape(-1)[:EPC])
    kernel.last_results = res
    return np.concatenate(outs)


# revision 14
# speedup vs baseline: 1.0216x; 1.0216x over previous
"""Trainium2 Bass kernel for nn_CombineLayer (gnn_message_passing).

Math: out_e = z[i0]@wo1 + z[i1]@v2 + z0[max(i0,i1)-M]@v3 + bo, where
wo1 = Wo[0,:128], v2 = Wq[:,:128].T @ Wo[0,128:], v3 = Wq[:,128:].T @ Wo[0,128:]
and M = min_e max(i0,i1). (Exact rewrite of the reference's two linears.)

Device work per core (edges sharded 8 ways, node tables sharded 8 ways):
  1. tables: a = z@wo1, b = z@v2, c = z0_shift@v3 over this core's node
     shard (z passed transposed so matmuls need no on-chip transpose),
     assembled into 4 segments [a+bo, a+c+bo, b, b+c].
  2. AllGather the 8 table shards -> full table G (400k entries).
  3. per edge: out = G[pos0] + G[pos1] via 4096-index indirect DMA
     gathers; the second stream accumulates with the DMA CCE adder.
Host does only integer index prep / layout (sharding, transpose,
position computation), all float math runs on device.
"""
import sys

sys.path.insert(0, "/opt/trn_rl_repo")

import numpy as np

N = 100000
E = 1000000
EMB = 128
NCORES = 8

S = 12500            # nodes per core (table shard)
SP = 12544           # padded to 128*98
TT = 98              # node tiles of 128 per core
SEG = 4              # table segments per core
RANK_BLK = SEG * SP  # 50176 table entries per rank
TBL = NCORES * RANK_BLK  # 401408

EPC = 125000         # edges per core
NIDX = 4096          # indices per gather instruction
CHUNKS = 31          # ceil(EPC / NIDX)
EPAD = CHUNKS * NIDX  # 126976


def _build_nc():
    import concourse.bass as bass
    import concourse.bacc as bacc
    import concourse.mybir as mybir
    import concourse.tile as tile

    nc = bacc.Bacc("TRN2", target_bir_lowering=False, debug=False,
                   num_devices=NCORES)
    f32 = mybir.dt.float32
    zT = nc.dram_tensor("zT", [EMB, SP], f32, kind="ExternalInput").ap()
    z0T = nc.dram_tensor("z0T", [EMB, SP], f32, kind="ExternalInput").ap()
    Wq = nc.dram_tensor("Wq", [EMB, 2 * EMB], f32, kind="ExternalInput").ap()
    Wo = nc.dram_tensor("Wo", [1, 2 * EMB], f32, kind="ExternalInput").ap()
    bo = nc.dram_tensor("bo", [1, 1], f32, kind="ExternalInput").ap()
    idx0 = nc.dram_tensor("idx0", [128, 32 * CHUNKS], mybir.dt.int32,
                          kind="ExternalInput").ap()
    idx1 = nc.dram_tensor("idx1", [128, 32 * CHUNKS], mybir.dt.int32,
                          kind="ExternalInput").ap()
    out = nc.dram_tensor("out", [CHUNKS, NIDX], f32,
                         kind="ExternalOutput").ap()

    with tile.TileContext(nc) as tc:
        with (
            tc.tile_pool(name="sb", bufs=2) as sb,
            tc.tile_pool(name="ps", bufs=2, space="PSUM") as ps,
            tc.tile_pool(name="dram", bufs=1, space="DRAM") as dp,
            tc.tile_pool(name="tbl", bufs=1) as tp,
        ):
            # ---- load weights / build projection vectors ----
            wq_s = sb.tile([128, 2 * EMB], f32, bufs=1)
            nc.sync.dma_start(wq_s[:], Wq[:])
            # wo columns: Wo[0, 0:128] and Wo[0, 128:256] as [128, 1]
            wo1_s = sb.tile([128, 1], f32, bufs=1)
            nc.sync.dma_start(
                wo1_s[:], Wo[0:1, 0:EMB].rearrange("a (p b) -> (a p) b", b=1))
            wo2_s = sb.tile([128, 1], f32, bufs=1)
            nc.sync.dma_start(
                wo2_s[:], Wo[0:1, EMB:2 * EMB].rearrange("a (p b) -> (a p) b",
                                                         b=1))
            bo_s = sb.tile([1, 1], f32, bufs=1)
            nc.sync.dma_start(bo_s[:], bo[:])
            bo128 = sb.tile([128, 1], f32, bufs=1)
            nc.gpsimd.partition_broadcast(bo128[:], bo_s[:])

            v2_p = ps.tile([128, 1], f32)
            nc.tensor.matmul(v2_p[:], wq_s[:, 0:EMB], wo2_s[:],
                             start=True, stop=True)
            v3_p = ps.tile([128, 1], f32)
            nc.tensor.matmul(v3_p[:], wq_s[:, EMB:2 * EMB], wo2_s[:],
                             start=True, stop=True)
            # W_ab = [wo1 | v2], W_c = [v3]
            w_ab = sb.tile([128, 2], f32, bufs=1)
            nc.vector.tensor_copy(w_ab[:, 0:1], wo1_s[:])
            nc.vector.tensor_copy(w_ab[:, 1:2], v2_p[:])
            w_c = sb.tile([128, 1], f32, bufs=1)
            nc.vector.tensor_copy(w_c[:], v3_p[:])

            # ---- node tables for this core's shard ----
            ab_p = ps.tile([128, 2 * TT], f32, bufs=1)   # a,b interleaved
            c_p = ps.tile([128, TT], f32, bufs=1)
            CW = 14  # node tiles per load chunk
            for c in range(TT // CW):
                zc = tp.tile([128, 128 * CW], f32, bufs=2, tag="zz")
                nc.sync.dma_start(zc[:], zT[:, 128 * CW * c:128 * CW * (c + 1)])
                for j in range(CW):
                    t = c * CW + j
                    nc.tensor.matmul(ab_p[:, 2 * t:2 * t + 2],
                                     zc[:, 128 * j:128 * (j + 1)], w_ab[:],
                                     start=True, stop=True)
                z0c = tp.tile([128, 128 * CW], f32, bufs=2, tag="zz")
                nc.sync.dma_start(z0c[:],
                                  z0T[:, 128 * CW * c:128 * CW * (c + 1)])
                for j in range(CW):
                    t = c * CW + j
                    nc.tensor.matmul(c_p[:, t:t + 1],
                                     z0c[:, 128 * j:128 * (j + 1)], w_c[:],
                                     start=True, stop=True)

            a_view = ab_p[:].rearrange("p (t two) -> p t two", two=2)[:, :, 0:1]
            b_view = ab_p[:].rearrange("p (t two) -> p t two", two=2)[:, :, 1:2]
            seg1a = sb.tile([128, TT], f32, bufs=1)
            nc.vector.tensor_scalar_add(
                seg1a[:].rearrange("p (t o) -> p t o", o=1), a_view, bo128[:, 0:1])
            seg1b = sb.tile([128, TT], f32, bufs=1)
            nc.vector.tensor_add(seg1b[:], seg1a[:], c_p[:])
            seg2a = sb.tile([128, TT], f32, bufs=1)
            nc.vector.tensor_copy(
                seg2a[:].rearrange("p (t o) -> p t o", o=1), b_view)
            seg2b = sb.tile([128, TT], f32, bufs=1)
            nc.vector.tensor_add(seg2b[:], seg2a[:], c_p[:])

            # ---- AllGather table shards ----
            ag_in = dp.tile([1, RANK_BLK], f32)
            for si, seg in enumerate([seg1a, seg1b, seg2a, seg2b]):
                nc.sync.dma_start(
                    ag_in[0:1, si * SP:(si + 1) * SP].rearrange(
                        "a (p t) -> (a p) t", p=128),
                    seg[:])
            g_tbl = dp.tile([1, TBL], f32, addr_space="Shared")
            nc.gpsimd.collective_compute(
                "AllGather",
                mybir.AluOpType.bypass,
                replica_groups=[list(range(NCORES))],
                ins=[ag_in.opt()],
                outs=[g_tbl.opt()],
            )
            g_src = g_tbl[:].rearrange("a (n b) -> (a n) b", b=1)

            # ---- edge gathers: out = G[pos0] + G[pos1] ----
            it0 = sb.tile([128, 32 * CHUNKS], mybir.dt.int32, bufs=1)
            nc.sync.dma_start(it0[:], idx0[:])
            it1 = sb.tile([128, 32 * CHUNKS], mybir.dt.int32, bufs=1)
            nc.sync.dma_start(it1[:], idx1[:])
            for ch in range(CHUNKS):
                g0 = sb.tile([1, NIDX], f32, tag="g0", bufs=4)
                nc.gpsimd.indirect_dma_start(
                    out=g0[:].rearrange("p (a b) -> p a b", b=1),
                    out_offset=None, in_=g_src,
                    in_offset=bass.IndirectOffsetOnAxis(
                        ap=it0[:, 32 * ch:32 * ch + 32], axis=0))
                g1 = sb.tile([1, NIDX], f32, tag="g1", bufs=4)
                nc.gpsimd.indirect_dma_start(
                    out=g1[:].rearrange("p (a b) -> p a b", b=1),
                    out_offset=None, in_=g_src,
                    in_offset=bass.IndirectOffsetOnAxis(
                        ap=it1[:, 32 * ch:32 * ch + 32], axis=0))
                go = sb.tile([1, NIDX], f32, tag="go", bufs=2)
                nc.vector.tensor_add(go[:], g0[:], g1[:])
                nc.sync.dma_start(out[ch:ch + 1, :], go[:])
    nc.compile()
    return nc


def kernel(z, edge_index, z0, Wq, Wo, bo):
    z = np.asarray(z, dtype=np.float32)
    z0 = np.asarray(z0, dtype=np.float32)
    Wq = np.asarray(Wq, dtype=np.float32)
    Wo = np.asarray(Wo, dtype=np.float32)
    bo = np.asarray(bo, dtype=np.float32)
    ei = np.asarray(edge_index)

    i0 = ei[0].astype(np.int64)
    i1 = ei[1].astype(np.int64)
    m = np.maximum(i0, i1)
    M = int(m.min())

    # table positions (int index preprocessing only)
    def pos_of(i_end, seg_pair_base, shifted):
        k = i_end // S
        n_loc = i_end - k * S
        t = n_loc // 128
        p = n_loc - t * 128
        seg = seg_pair_base + shifted
        return k * RANK_BLK + seg * SP + p * TT + t

    sh0 = (i0 >= i1).astype(np.int64)     # table1 shifted half used
    sh1 = 1 - sh0                         # table2 shifted half used
    pos0 = pos_of(i0, 0, sh0).astype(np.int32)
    pos1 = pos_of(i1, 2, sh1).astype(np.int32)

    # per-core inputs
    in_maps = []
    for k in range(NCORES):
        # node shard rows [k*S, k*S+SP) of z; z0 shifted window rows - M
        zpad = np.zeros((SP, EMB), dtype=np.float32)
        lo, hi = k * S, min(k * S + SP, N)
        zpad[:hi - lo] = z[lo:hi]
        z0pad = np.zeros((SP, EMB), dtype=np.float32)
        wlo, whi = k * S - M, k * S + SP - M
        slo, shi = max(wlo, 0), min(whi, N)
        if shi > slo:
            z0pad[slo - wlo:shi - wlo] = z0[slo:shi]
        p0 = np.zeros(EPAD, dtype=np.int32)
        p1 = np.zeros(EPAD, dtype=np.int32)
        p0[:EPC] = pos0[k * EPC:(k + 1) * EPC]
        p1[:EPC] = pos1[k * EPC:(k + 1) * EPC]
        # snake layout: instruction ch consumes idx[p, 32ch + c] with
        # linear order i = c*128 + p
        it0 = p0.reshape(CHUNKS, 32, 128).transpose(2, 0, 1).reshape(
            128, CHUNKS * 32)
        it1 = p1.reshape(CHUNKS, 32, 128).transpose(2, 0, 1).reshape(
            128, CHUNKS * 32)
        in_maps.append({
            "zT": np.ascontiguousarray(zpad.T),
            "z0T": np.ascontiguousarray(z0pad.T),
            "Wq": Wq, "Wo": Wo, "bo": bo.reshape(1, 1),
            "idx0": np.ascontiguousarray(it0),
            "idx1": np.ascontiguousarray(it1),
        })

    from concourse.bass_utils import run_bass_kernel_spmd
    nc = _build_nc()
    res = run_bass_kernel_spmd(nc, in_maps, core_ids=list(range(NCORES)))
    outs = []
    for k in range(NCORES):
        outs.append(res.results[k]["out"].reshape(-1)[:EPC])
    kernel.last_results = res
    return np.concatenate(outs)
